# revision 29
# baseline (speedup 1.0000x reference)
"""Trainium2 Bass kernel for CP-decomposed conv2d (nn_CPDConvolution2D).

Reference computation (NCHW, fp32):
  h = conv1x1(x, W1)         [N,64,224,224] -> [N,32,224,224]
  h = depthwise 3x1 vertical (pad 1)
  h = depthwise 1x3 horizontal (pad 1)
  y = conv1x1(h, W4) + bias  -> [N,128,224,224]

Sharding: data-parallel over batch, 2 images per core on 8 cores.

This version is HBM-bandwidth-driven: the fp32 pipe moved 80 MB/core
(read x + write y) against a ~358 GB/s/core HBM cap, so all IO and
SBUF-side compute run in fp16 (x is cast on the host, y is written
fp16 and cast back on the host; PSUM accumulation stays fp32 in HW).
Rel-err budget is 2e-2; fp16 end-to-end costs ~1e-3.

Per-core layout: images are processed in 7 strips of HB=32 rows, each
strip split over 4 row groups of GB=8 rows on PSUM partition quadrants.

The vertical 3x1 depthwise conv is FOLDED into stage A: for tap kv,
lhsT_kv = (diag(wv[:,kv]) @ W1).T, and the three taps accumulate in
PSUM over row-shifted x reads.  Stage A therefore directly produces
the vertically-convolved h2 (GB rows per group, no halo rows), and the
old DVE vertical pass + 2 extra copy rows disappear.  The horizontal
1x3 conv stays on DVE/GpSimd in fp16 (taps kh=0,2 hit the packed
2x/4x modes; the odd-offset kh=1 tap runs on otherwise-idle GpSimd).

Stage B is unchanged (row-tiled 1x1 expansion) but writes into 2-bank
PSUM pair tiles so each bias+copy instruction covers two groups
(FD=896), split between ACT and DVE to balance the PSUM->SBUF flux.
"""
import os
import sys
import types

sys.path.insert(0, '/opt/trn_rl_repo')

import numpy as np

import concourse.bass as bass
import concourse.mybir as mybir
from concourse.tile import TileContext

# ---------------------------------------------------------------------------
# Environment compat: NTFF profile hook (for trace timing) and a sync
# legalizer for this container's walrus build, which accepts at most one
# sem wait and one sem update per instruction while Tile attaches several
# at dependency joins.
# ---------------------------------------------------------------------------


def _install_ntff_hook():
    if "antenv.axon_hooks" in sys.modules:
        return
    try:
        from trn_agent_boot.trn_boot import _ntff_profile_via_ctypes
    except ImportError:
        return
    _hook = _ntff_profile_via_ctypes('/opt/axon/libaxon_pjrt.so')
    m = types.ModuleType("antenv.axon_hooks")
    m.get_axon_ntff_profile_hook = lambda: _hook
    m.set_axon_ntff_profile_hook = lambda h: None
    sys.modules["antenv.axon_hooks"] = m
    from concourse import bass_utils
    bass_utils.upload_artifacts = lambda tmpdir: "local://" + tmpdir
    if int(os.environ.get("KERNEL_LDWOPT", "0")):
        # flip walrus's disabled LDWEIGHTS-elision pass back on: ~half
        # the PE queue entries are weight reloads, many of them
        # identical back-to-back in tap-major emission.
        _orig_run_command = bass_utils.run_command

        def _patched_run_command(argv, **kwargs):
            argv = ["--enable-ldw-opt=true" if a == "--enable-ldw-opt=false"
                    else a for a in argv]
            return _orig_run_command(argv, **kwargs)

        bass_utils.run_command = _patched_run_command


def _legalize_sync(nc):
    """Split multi-wait/multi-update instructions onto same-engine NoOps.

    Engine queues execute in order, so waits hoisted onto NoOps placed
    before an instruction still gate it; an update pushed onto a NoOp
    after a compute instruction fires only once that instruction has
    completed (the documented-safe `op; nop().then_inc(sem)` idiom).
    Moving a DMA's completion update is NOT safe -- assert instead.
    """
    for f in nc.m.functions:
        for bb in f.blocks:
            idx = 0
            while idx < len(bb.instructions):
                inst = bb.instructions[idx]
                si = inst.sync_info
                if si is None:
                    idx += 1
                    continue
                waits = si.on_wait
                if waits is not None and len(waits) > 1:
                    extra = list(waits[:-1])
                    del si.on_wait[:-1]
                    for w in extra:
                        nop = mybir.InstNoOp(
                            name=nc.get_next_instruction_name(),
                            engine=inst.engine, ins=[], outs=[],
                        )
                        nop.sync_info = mybir.SyncInfo(on_wait=[w], on_update=[])
                        nc.register_instruction(nop)
                        bb.instructions.insert(idx, nop)
                        idx += 1
                    si = inst.sync_info
                upds = si.on_update
                if upds is not None and len(upds) > 1:
                    assert not isinstance(
                        inst,
                        (mybir.InstDMACopy, mybir.InstDMA, mybir.InstDmaTransposeAnt),
                    ), f"multi-update on DMA instruction {inst.name}"
                    extra = list(upds[1:])
                    del si.on_update[1:]
                    for u in extra:
                        nop = mybir.InstNoOp(
                            name=nc.get_next_instruction_name(),
                            engine=inst.engine, ins=[], outs=[],
                        )
                        nop.sync_info = mybir.SyncInfo(on_wait=[], on_update=[u])
                        nc.register_instruction(nop)
                        bb.instructions.insert(idx + 1, nop)
                idx += 1


# ---------------------------------------------------------------------------
# Problem shapes (hardcoded per spec)
# ---------------------------------------------------------------------------
N_FULL, S_CH, H_IMG, W_IMG = 16, 64, 224, 224
R_CH, T_CH = 32, 128
N_CORES = 8
N_PER_CORE = N_FULL // N_CORES     # 2 images per core
HB = int(os.environ.get("KERNEL_HB", "32"))  # strip height (rows)
GB = HB // 4                       # rows per partition group
N_STRIPS = H_IMG // HB
XR = 2 * GB + 2                    # x rows per partition half per strip
FP32 = mybir.dt.float32
FP16 = mybir.dt.float16

# Of the 8 bias+copy pair ops per strip, how many go to ACT (rest DVE).
BIAS_ACT = int(os.environ.get("KERNEL_BIAS_ACT", "4"))
# Engine for the odd-offset (1x-rate) horizontal tap: gpsimd frees DVE.
# (TensorScalarPtr is rejected on Pool by this walrus build, so default off.)
GPS_TAP = bool(int(os.environ.get("KERNEL_GPS_TAP", "0")))
# Fold the horizontal 1x3 conv into stage B as 3 PSUM-accumulated matmuls
# with tap-scaled W4 weights (eliminates the DVE tap ops entirely).
HFOLD = bool(int(os.environ.get("KERNEL_HFOLD", "0")))
# Emit stage-A taps tap-major (groups interleaved between taps).  Only
# sound if a start=True matmul clears PSUM written-bits for its own
# region rather than the whole bank -- verified empirically by rel-err.
A_TAPMAJOR = bool(int(os.environ.get("KERNEL_A_TAPMAJOR", "0")))
# Split each strip's store into two half-stores to start draining early.
SPLIT_STORE = bool(int(os.environ.get("KERNEL_SPLIT_STORE", "0")))
# Halo-free loads: xh is a pure reshape of x (no duplicated halo rows);
# strip-edge taps read the neighbor strip's tile (zero row at image
# edges), splitting the 4 boundary taps per strip into 1-row matmuls.
NOHALO = bool(int(os.environ.get("KERNEL_NOHALO", "0")))
# Emit B(t-1) before A(t) within each chunk iteration (prioritizes the
# store-feeding path in the PE queue).
B_FIRST = bool(int(os.environ.get("KERNEL_B_FIRST", "0")))
XRN = HB // 2                      # x rows per partition half (halo-free)

_CACHE = {}
LAST_EXEC_TIME_NS = None
LAST_RES = None


def _build_nc():
    nc = bass.Bass(target_bir_lowering=False)

    # x pre-arranged on host into half-major strip tiles: partition
    # p<64 holds channel p rows [h0-1, h0+17), p>=64 holds channel
    # p-64 rows [h0+15, h0+33) (image edges zero-padded).  One strip =
    # one contiguous 128-partition DMA -> all 16 SDMA engines.
    xh = nc.dram_tensor(
        "xh", [N_PER_CORE, N_STRIPS, 128, XRN if NOHALO else XR, W_IMG],
        FP16, kind="ExternalInput")
    # (diag(wv[:,kv]) @ W1).T per tap, stacked twice over partitions so
    # groups 2-3 can source it at partition base 64: [128, 3, 32]
    w1vT = nc.dram_tensor("w1vT", [2 * S_CH, 3, R_CH], FP16,
                          kind="ExternalInput")
    if HFOLD:
        # (W4 @ diag-per-tap of wh).T per tap: [128, 3, 128]
        w4hs = nc.dram_tensor("w4hs", [128, 3, 128], FP16,
                              kind="ExternalInput")
    else:
        wh = nc.dram_tensor("wh", [128, 3], FP32, kind="ExternalInput")
        w4s = nc.dram_tensor("w4s", [128, 128], FP16, kind="ExternalInput")
    bias = nc.dram_tensor("bias", [128, 1], FP32, kind="ExternalInput")
    y = nc.dram_tensor("y", [N_PER_CORE, T_CH, H_IMG, W_IMG], FP16,
                       kind="ExternalOutput")

    # spread the ACT-assigned bias pair ops evenly over the strip
    npairs = GB                     # 2 pairs per c-step, GB/2 c-steps
    n_act = max(0, min(npairs, round(BIAS_ACT * npairs / 8)))
    act_set = {int(i * npairs / n_act) for i in range(n_act)} if n_act else set()

    with TileContext(nc) as tc:
        with (
            tc.tile_pool(name="consts", bufs=1) as consts,
            tc.tile_pool(name="xin", bufs=4) as xin,
            tc.tile_pool(name="mid", bufs=3 if HFOLD else 2) as mid,
            tc.tile_pool(name="h3pool", bufs=1 if HFOLD else 3) as h3pool,
            tc.tile_pool(name="oout", bufs=3) as oout,
            tc.tile_pool(name="psA", bufs=int(os.environ.get("KERNEL_PSA", "2")),
                         space="PSUM") as psumA,
            tc.tile_pool(name="psB", bufs=int(os.environ.get("KERNEL_PSB", "3")),
                         space="PSUM") as psumB,
        ):
            w1vT_t = consts.tile([2 * S_CH, 3, R_CH], FP16)
            bias_t = consts.tile([128, 1], FP32)
            zrow_t = consts.tile([128, 1, W_IMG], FP16)
            nc.gpsimd.memset(zrow_t[:, :, :], 0.0)
            nc.sync.dma_start(out=w1vT_t[:], in_=w1vT[:, :, :])
            nc.sync.dma_start(out=bias_t[:], in_=bias[:, :])
            if HFOLD:
                w4hs_t = consts.tile([128, 3, 128], FP16)
                nc.sync.dma_start(out=w4hs_t[:], in_=w4hs[:, :, :])
            else:
                wh_t = consts.tile([128, 3], FP32)
                w4s_t = consts.tile([128, 128], FP16)
                nc.sync.dma_start(out=wh_t[:], in_=wh[:, :])
                nc.sync.dma_start(out=w4s_t[:], in_=w4s[:, :])

            N_TOT = N_PER_CORE * N_STRIPS
            live = {}

            def load_x(t):
                n, s = divmod(t, N_STRIPS)
                x_t = xin.tile([128, XRN if NOHALO else XR, W_IMG], FP16)
                live[("x", t)] = x_t
                nc.gpsimd.dma_start(out=x_t[:, :, :], in_=xh[n, s, :, :, :])

            def a_step(t, c):
                # ---- stage A chunk-step: 1x1 S->R with the vertical
                # 3x1 depthwise folded in as 3 PSUM-accumulated taps.
                # Groups are col-tiled x4 on the PE; groups 0-1 contract
                # x from partitions 0-63, groups 2-3 from 64-127.
                # Emission is group-major (taps innermost) so each
                # group's start..stop accumulation run is contiguous --
                # a start=True matmul clears written-bits bank-wide, so
                # interleaving groups' taps could drop accumulation.
                x_t = live[("x", t)]
                if c == 0:
                    h2p = mid.tile([128, GB, W_IMG + 2], FP16, tag="h2p",
                                   name=f"h2p_{t}")
                    live[("h2p", t)] = h2p
                    nc.gpsimd.memset(h2p[:, :, 0:1], 0.0)
                    nc.gpsimd.memset(h2p[:, :, W_IMG + 1:W_IMG + 2], 0.0)
                h2p = live[("h2p", t)]
                psA = psumA.tile([128, 2, W_IMG], FP32)
                order = ([(j, kv) for kv in range(3) for j in range(4)]
                         if A_TAPMAJOR else
                         [(j, kv) for j in range(4) for kv in range(3)])
                n_img, s = divmod(t, N_STRIPS)

                def emit(j, kv, rhs, base, drow, nrows):
                    nc.tensor.matmul(
                        psA[32 * j:32 * j + 32, drow:drow + nrows, :],
                        w1vT_t[base:base + S_CH, kv:kv + 1, :], rhs,
                        start=(kv == 0), stop=(kv == 2),
                        tile_position=(base, 32 * j),
                    )

                def src_row(i):
                    # one x row i (strip-local, in [-1, HB]) -> (tile,
                    # partition base, local row); image edges -> zero row
                    if i < 0:
                        if s == 0:
                            return (zrow_t, 64, 0)
                        return (live[("x", t - 1)], 64, XRN - 1)
                    if i >= HB:
                        if s == N_STRIPS - 1:
                            return (zrow_t, 0, 0)
                        return (live[("x", t + 1)], 0, 0)
                    if i < XRN:
                        return (x_t, 0, i)
                    return (x_t, 64, i - XRN)

                for j, kv in order:
                    if not NOHALO:
                        if j < 2:
                            r0 = GB * j + 2 * c + kv
                            rhs = x_t[0:S_CH, r0:r0 + 2, :]
                            base = 0
                        else:
                            r0 = GB * j + 2 * c + kv - 2 * GB
                            rhs = x_t[S_CH:128, r0:r0 + 2, :]
                            base = 64
                        emit(j, kv, rhs, base, 0, 2)
                        continue
                    i = GB * j + 2 * c + kv - 1
                    ta, ba, la = src_row(i)
                    tb, bb, lb = src_row(i + 1)
                    if ta is tb and ba == bb and lb == la + 1:
                        emit(j, kv, ta[ba:ba + S_CH, la:la + 2, :], ba, 0, 2)
                    else:
                        emit(j, kv, ta[ba:ba + S_CH, la:la + 1, :], ba, 0, 1)
                        emit(j, kv, tb[bb:bb + S_CH, lb:lb + 1, :], bb, 1, 1)
                nc.scalar.copy(h2p[:, 2 * c:2 * c + 2, 1:W_IMG + 1],
                               psA[:, :, :])

            def depthwise_h(t):
                # ---- horizontal 1x3 depthwise, batched over the strip.
                # h2p data sits at cols 1..224 with zero pads at 0 and
                # 225, so tap kh reads cols [kh, kh+224).  kh=0/2 are
                # 4B-aligned (fp16 packed modes); the odd-offset kh=1
                # tap is 1x-rate and can run on GpSimd instead.
                if NOHALO:
                    if t >= 1:
                        live.pop(("x", t - 1))
                else:
                    live.pop(("x", t))
                if HFOLD:
                    return
                h2p = live.pop(("h2p", t))
                h3 = h3pool.tile([128, GB, W_IMG], FP16, tag="h3",
                                 name=f"h3_{t}")
                nc.vector.tensor_scalar_mul(
                    h3[:, :, :], h2p[:, :, 0:W_IMG], wh_t[:, 0:1])
                eng1 = nc.gpsimd if GPS_TAP else nc.vector
                eng1.scalar_tensor_tensor(
                    h3[:, :, :], h2p[:, :, 1:W_IMG + 1], wh_t[:, 1:2],
                    h3[:, :, :],
                    op0=mybir.AluOpType.mult, op1=mybir.AluOpType.add)
                nc.vector.scalar_tensor_tensor(
                    h3[:, :, :], h2p[:, :, 2:W_IMG + 2], wh_t[:, 2:3],
                    h3[:, :, :],
                    op0=mybir.AluOpType.mult, op1=mybir.AluOpType.add)
                live[("h3", t)] = h3

            def b_step(t, c):
                # ---- stage B chunk-step: 1x1 R->T row-tiled x4 into
                # 2-bank PSUM pair tiles; one bias+copy op per pair.
                # With HFOLD, each group runs 3 PSUM-accumulated taps
                # whose rhs is h2p column-shifted by kh (the horizontal
                # conv rides the contraction); each pair member owns its
                # own PSUM bank, so interleaved tap emission is safe.
                if c == 0:
                    # [T, group, row-in-group, W] == y rows g*8+r
                    o_t = oout.tile([T_CH, 4, GB, W_IMG], FP16, tag="o_t",
                                    name=f"o_t_{t}")
                    live[("o", t)] = o_t
                o_t = live[("o", t)]
                src = live[("h2p", t)] if HFOLD else live[("h3", t)]
                psBs = [psumB.tile([128, 2, 512], FP32, tag="psB",
                                   name=f"psB_{t}_{c}_{k}")
                        for k in range(2)]
                if HFOLD:
                    # tap-major across all 4 groups: each group owns its
                    # own PSUM bank, so start-flag bit-clears can't
                    # interfere, and consecutive MMs hit independent PE
                    # tiles (pipeline overlap instead of 3-MM chains).
                    for kh in range(3):
                        for g in range(4):
                            nc.tensor.matmul(
                                psBs[g // 2][:, g % 2:g % 2 + 1, 0:2 * W_IMG],
                                w4hs_t[32 * g:32 * g + 32, kh:kh + 1, :],
                                src[32 * g:32 * g + 32, 2 * c:2 * c + 2,
                                    kh:kh + W_IMG],
                                start=(kh == 0), stop=(kh == 2),
                                tile_position=(32 * g, 0),
                            )
                else:
                    for g in range(4):
                        nc.tensor.matmul(
                            psBs[g // 2][:, g % 2:g % 2 + 1, 0:2 * W_IMG],
                            w4s_t[32 * g:32 * g + 32, :],
                            src[32 * g:32 * g + 32, 2 * c:2 * c + 2, :],
                            start=True, stop=True,
                            tile_position=(32 * g, 0),
                        )
                for k2 in range(2):
                    out_ap = o_t[:, 2 * k2:2 * k2 + 2, 2 * c:2 * c + 2, :]
                    in_ap = psBs[k2][:, :, 0:2 * W_IMG]
                    if (2 * c + k2) in act_set:
                        nc.scalar.add(out_ap, in_ap, bias_t[:, 0:1])
                    else:
                        nc.vector.tensor_scalar_add(out_ap, in_ap,
                                                    bias_t[:, 0:1])

            def b_dma(t, half):
                # optionally store rows [0,GB/2) after c-step 1 and the
                # rest after c-step 3, so the drain starts early.
                n, s = divmod(t, N_STRIPS)
                h0 = s * HB
                o_t = live[("o", t)]
                if not SPLIT_STORE and half == 0:
                    return
                # stores ride the scalar HWDGE ring so reads (sync ring)
                # and writes overlap instead of FIFO-ing on one queue
                eng = nc.scalar if t % 2 == 0 else nc.sync
                if SPLIT_STORE:
                    r0 = half * (GB // 2)
                    yv = y[n, :, h0:h0 + HB, :].rearrange(
                        "p (g r) w -> p g r w", g=4)
                    eng.dma_start(
                        out=yv[:, :, r0:r0 + GB // 2, :],
                        in_=o_t[:, :, r0:r0 + GB // 2, :])
                else:
                    eng.dma_start(out=y[n, :, h0:h0 + HB, :],
                                  in_=o_t[:, :, :, :])
                if half == 1:
                    live.pop(("o", t))
                    live.pop(("h2p", t) if HFOLD else ("h3", t))

            # One-strip-pipelined with a SKEW-strip skew for stage B.
            SKEW = 1 if HFOLD else 2
            NC_CH = GB // 2         # 4 chunk-steps for both stages
            for t in range(N_TOT + SKEW):
                if t < N_TOT:
                    if t == 0:
                        load_x(0)
                    if t + 1 < N_TOT:
                        load_x(t + 1)
                    for c in range(NC_CH):
                        if B_FIRST and t >= SKEW:
                            b_step(t - SKEW, c)
                        a_step(t, c)
                        if t >= SKEW:
                            if not B_FIRST:
                                b_step(t - SKEW, c)
                            if c == 1:
                                b_dma(t - SKEW, 0)
                    if t >= SKEW:
                        b_dma(t - SKEW, 1)
                    depthwise_h(t)
                else:
                    for c in range(NC_CH):
                        b_step(t - SKEW, c)
                        if c == 1:
                            b_dma(t - SKEW, 0)
                    b_dma(t - SKEW, 1)

    _legalize_sync(nc)
    return nc


def _prep_weights(s_to_r_weight, depth_vert_weight, depth_hor_weight,
                  r_to_t_weight, r_to_t_bias):
    W1 = s_to_r_weight[:, :, 0, 0].astype(np.float32)          # [32, 64]
    wv = depth_vert_weight[:, 0, :, 0].astype(np.float32)      # [32, 3]
    # lhsT per tap: (diag(wv[:,kv]) @ W1).T = [64, 32]; -> [64, 3, 32]
    w1vT = np.stack([(W1 * wv[:, kv:kv + 1]).T for kv in range(3)],
                    axis=1)
    w1vT = np.ascontiguousarray(
        np.tile(w1vT, (2, 1, 1)).astype(np.float16))           # [128, 3, 32]
    wh = np.ascontiguousarray(
        np.tile(depth_hor_weight[:, 0, 0, :], (4, 1)).astype(np.float32))
    W4T = r_to_t_weight[:, :, 0, 0].T.astype(np.float32)       # [32, 128]
    w4s = np.ascontiguousarray(np.tile(W4T, (4, 1)).astype(np.float16))
    wh32 = depth_hor_weight[:, 0, 0, :].astype(np.float32)     # [32, 3]
    # per-tap (W4 @ diag(wh[:,kh])).T = diag(wh[:,kh]) @ W4.T: [32, 3, 128]
    w4hs = np.stack([W4T * wh32[:, kh:kh + 1] for kh in range(3)], axis=1)
    w4hs = np.ascontiguousarray(
        np.tile(w4hs, (4, 1, 1)).astype(np.float16))           # [128, 3, 128]
    b = np.ascontiguousarray(
        r_to_t_bias.reshape(T_CH, 1).astype(np.float32))
    return w1vT, wh, w4s, w4hs, b


def kernel(x, s_to_r_weight, depth_vert_weight, depth_hor_weight,
           r_to_t_weight, r_to_t_bias):
    global LAST_EXEC_TIME_NS, LAST_RES
    _install_ntff_hook()
    from concourse.bass_utils import run_bass_kernel_spmd

    if "nc" not in _CACHE:
        _CACHE["nc"] = _build_nc()
    nc = _CACHE["nc"]

    x16 = np.asarray(x, dtype=np.float16)
    if NOHALO:
        # pure reshape: [N, S, strip, half, XRN, W] -> [N, strip, 128, XRN, W]
        xh_full = np.ascontiguousarray(
            x16.reshape(N_FULL, S_CH, N_STRIPS, 2, XRN, W_IMG)
               .transpose(0, 2, 3, 1, 4, 5)
               .reshape(N_FULL, N_STRIPS, 128, XRN, W_IMG))
    else:
        # half-major strip tiles with materialized halo rows; edges 0
        xpad = np.zeros((N_FULL, S_CH, H_IMG + 2, W_IMG), dtype=np.float16)
        xpad[:, :, 1:H_IMG + 1] = x16
        sv = np.lib.stride_tricks.sliding_window_view(
            xpad, XR, axis=2)                   # [N,S,H+3-XR,W,XR]
        idx0 = np.arange(N_STRIPS) * HB         # top half: rows h0-1 (+1 pad)
        idx1 = idx0 + 2 * GB                    # bottom half
        h0v = sv[:, :, idx0].transpose(0, 2, 1, 4, 3)
        h1v = sv[:, :, idx1].transpose(0, 2, 1, 4, 3)
        xh_full = np.ascontiguousarray(
            np.concatenate([h0v, h1v], axis=2))  # [N, strip, 128, XR, W]

    w1vT, wh, w4s, w4hs, b = _prep_weights(
        np.asarray(s_to_r_weight), np.asarray(depth_vert_weight),
        np.asarray(depth_hor_weight), np.asarray(r_to_t_weight),
        np.asarray(r_to_t_bias))

    in_maps = []
    for i in range(N_CORES):
        m = {
            "xh": xh_full[i * N_PER_CORE:(i + 1) * N_PER_CORE],
            "w1vT": w1vT, "bias": b,
        }
        if HFOLD:
            m["w4hs"] = w4hs
        else:
            m["wh"] = wh
            m["w4s"] = w4s
        in_maps.append(m)

    trace = bool(int(os.environ.get("KERNEL_TRACE", "0")))
    res = run_bass_kernel_spmd(nc, in_maps, core_ids=list(range(N_CORES)),
                               trace=trace)
    LAST_EXEC_TIME_NS = res.exec_time_ns
    LAST_RES = res

    out = np.empty((N_FULL, T_CH, H_IMG, W_IMG), dtype=np.float32)
    for i in range(N_CORES):
        out[i * N_PER_CORE:(i + 1) * N_PER_CORE] = \
            res.results[i]["y"].astype(np.float32)
    return out


# revision 30
# speedup vs baseline: 1.0020x; 1.0020x over previous
"""Trainium2 Bass kernel for CP-decomposed conv2d (nn_CPDConvolution2D).

Reference computation (NCHW, fp32):
  h = conv1x1(x, W1)         [N,64,224,224] -> [N,32,224,224]
  h = depthwise 3x1 vertical (pad 1)
  h = depthwise 1x3 horizontal (pad 1)
  y = conv1x1(h, W4) + bias  -> [N,128,224,224]

Sharding: data-parallel over batch, 2 images per core on 8 cores.

This version is HBM-bandwidth-driven: the fp32 pipe moved 80 MB/core
(read x + write y) against a ~358 GB/s/core HBM cap, so all IO and
SBUF-side compute run in fp16 (x is cast on the host, y is written
fp16 and cast back on the host; PSUM accumulation stays fp32 in HW).
Rel-err budget is 2e-2; fp16 end-to-end costs ~1e-3.

Per-core layout: images are processed in 7 strips of HB=32 rows, each
strip split over 4 row groups of GB=8 rows on PSUM partition quadrants.
x is pre-arranged on the host into half-major strip tiles (xh) so each
strip is ONE contiguous 128-partition DMA across all 16 SDMA engines.

BOTH depthwise convs are folded into the 1x1 matmuls as PSUM-
accumulated taps: stage A uses lhsT_kv = (diag(wv[:,kv]) @ W1).T over
row-shifted x reads (producing the vertically-convolved h2 directly),
and stage B uses lhsT_kh = (diag(wh[:,kh]) @ W4.T) over column-shifted
h2 reads.  The vector engine does nothing but its share of bias+copy.

Per strip: 48 A-matmuls (3 taps x 4 col-tiled groups x 4 two-row
chunks) -> ACT copies PSUM->SBUF h2 (fp16, W-padded); 48 B-matmuls
(3 taps x 4 row-tiled groups x 4 chunks) into 2-bank PSUM pair tiles
-> one bias+copy per pair (FD=896), split 3:5 ACT:DVE.  Stores
alternate the two HWDGE rings (scalar/sync) so per-ring FIFO HBM
write-completion latency (~1.7us/DMA) overlaps across strips; loads
ride the gpsimd SWDGE queue.  Tap emission is tap-major (verified:
PSUM start-flag bit-clear is region-scoped, so interleaving groups
between taps is sound and keeps 4 independent PE tiles in flight).
"""
import os
import sys
import types

sys.path.insert(0, '/opt/trn_rl_repo')

import numpy as np

import concourse.bass as bass
import concourse.mybir as mybir
from concourse.tile import TileContext

# ---------------------------------------------------------------------------
# Environment compat: NTFF profile hook (for trace timing) and a sync
# legalizer for this container's walrus build, which accepts at most one
# sem wait and one sem update per instruction while Tile attaches several
# at dependency joins.
# ---------------------------------------------------------------------------


def _install_ntff_hook():
    if "antenv.axon_hooks" in sys.modules:
        return
    try:
        from trn_agent_boot.trn_boot import _ntff_profile_via_ctypes
    except ImportError:
        return
    _hook = _ntff_profile_via_ctypes('/opt/axon/libaxon_pjrt.so')
    m = types.ModuleType("antenv.axon_hooks")
    m.get_axon_ntff_profile_hook = lambda: _hook
    m.set_axon_ntff_profile_hook = lambda h: None
    sys.modules["antenv.axon_hooks"] = m
    from concourse import bass_utils
    bass_utils.upload_artifacts = lambda tmpdir: "local://" + tmpdir
    if int(os.environ.get("KERNEL_LDWOPT", "0")):
        # flip walrus's disabled LDWEIGHTS-elision pass back on: ~half
        # the PE queue entries are weight reloads, many of them
        # identical back-to-back in tap-major emission.
        _orig_run_command = bass_utils.run_command

        def _patched_run_command(argv, **kwargs):
            argv = ["--enable-ldw-opt=true" if a == "--enable-ldw-opt=false"
                    else a for a in argv]
            return _orig_run_command(argv, **kwargs)

        bass_utils.run_command = _patched_run_command


def _legalize_sync(nc):
    """Split multi-wait/multi-update instructions onto same-engine NoOps.

    Engine queues execute in order, so waits hoisted onto NoOps placed
    before an instruction still gate it; an update pushed onto a NoOp
    after a compute instruction fires only once that instruction has
    completed (the documented-safe `op; nop().then_inc(sem)` idiom).
    Moving a DMA's completion update is NOT safe -- assert instead.
    """
    for f in nc.m.functions:
        for bb in f.blocks:
            idx = 0
            while idx < len(bb.instructions):
                inst = bb.instructions[idx]
                si = inst.sync_info
                if si is None:
                    idx += 1
                    continue
                waits = si.on_wait
                if waits is not None and len(waits) > 1:
                    extra = list(waits[:-1])
                    del si.on_wait[:-1]
                    for w in extra:
                        nop = mybir.InstNoOp(
                            name=nc.get_next_instruction_name(),
                            engine=inst.engine, ins=[], outs=[],
                        )
                        nop.sync_info = mybir.SyncInfo(on_wait=[w], on_update=[])
                        nc.register_instruction(nop)
                        bb.instructions.insert(idx, nop)
                        idx += 1
                    si = inst.sync_info
                upds = si.on_update
                if upds is not None and len(upds) > 1:
                    assert not isinstance(
                        inst,
                        (mybir.InstDMACopy, mybir.InstDMA, mybir.InstDmaTransposeAnt),
                    ), f"multi-update on DMA instruction {inst.name}"
                    extra = list(upds[1:])
                    del si.on_update[1:]
                    for u in extra:
                        nop = mybir.InstNoOp(
                            name=nc.get_next_instruction_name(),
                            engine=inst.engine, ins=[], outs=[],
                        )
                        nop.sync_info = mybir.SyncInfo(on_wait=[], on_update=[u])
                        nc.register_instruction(nop)
                        bb.instructions.insert(idx + 1, nop)
                idx += 1


# ---------------------------------------------------------------------------
# Problem shapes (hardcoded per spec)
# ---------------------------------------------------------------------------
N_FULL, S_CH, H_IMG, W_IMG = 16, 64, 224, 224
R_CH, T_CH = 32, 128
N_CORES = 8
N_PER_CORE = N_FULL // N_CORES     # 2 images per core
HB = int(os.environ.get("KERNEL_HB", "32"))  # strip height (rows)
GB = HB // 4                       # rows per partition group
N_STRIPS = H_IMG // HB
XR = 2 * GB + 2                    # x rows per partition half per strip
FP32 = mybir.dt.float32
FP16 = mybir.dt.float16

# Of the 8 bias+copy pair ops per strip, how many go to ACT (rest DVE).
BIAS_ACT = int(os.environ.get("KERNEL_BIAS_ACT", "3"))
# Engine for the odd-offset (1x-rate) horizontal tap: gpsimd frees DVE.
# (TensorScalarPtr is rejected on Pool by this walrus build, so default off.)
GPS_TAP = bool(int(os.environ.get("KERNEL_GPS_TAP", "0")))
# Fold the horizontal 1x3 conv into stage B as 3 PSUM-accumulated matmuls
# with tap-scaled W4 weights (eliminates the DVE tap ops entirely).
HFOLD = bool(int(os.environ.get("KERNEL_HFOLD", "1")))
# Emit stage-A taps tap-major (groups interleaved between taps).  Only
# sound if a start=True matmul clears PSUM written-bits for its own
# region rather than the whole bank -- verified empirically by rel-err.
A_TAPMAJOR = bool(int(os.environ.get("KERNEL_A_TAPMAJOR", "1")))
# Split each strip's store into two half-stores to start draining early.
SPLIT_STORE = bool(int(os.environ.get("KERNEL_SPLIT_STORE", "0")))
# Halo-free loads: xh is a pure reshape of x (no duplicated halo rows);
# strip-edge taps read the neighbor strip's tile (zero row at image
# edges), splitting the 4 boundary taps per strip into 1-row matmuls.
NOHALO = bool(int(os.environ.get("KERNEL_NOHALO", "0")))
# Emit B(t-1) before A(t) within each chunk iteration (prioritizes the
# store-feeding path in the PE queue).
B_FIRST = bool(int(os.environ.get("KERNEL_B_FIRST", "0")))
XRN = HB // 2                      # x rows per partition half (halo-free)

_CACHE = {}
LAST_EXEC_TIME_NS = None
LAST_RES = None


def _build_nc():
    nc = bass.Bass(target_bir_lowering=False)

    # x pre-arranged on host into half-major strip tiles: partition
    # p<64 holds channel p rows [h0-1, h0+17), p>=64 holds channel
    # p-64 rows [h0+15, h0+33) (image edges zero-padded).  One strip =
    # one contiguous 128-partition DMA -> all 16 SDMA engines.
    xh = nc.dram_tensor(
        "xh", [N_PER_CORE, N_STRIPS, 128, XRN if NOHALO else XR, W_IMG],
        FP16, kind="ExternalInput")
    # (diag(wv[:,kv]) @ W1).T per tap, stacked twice over partitions so
    # groups 2-3 can source it at partition base 64: [128, 3, 32]
    w1vT = nc.dram_tensor("w1vT", [2 * S_CH, 3, R_CH], FP16,
                          kind="ExternalInput")
    if HFOLD:
        # (W4 @ diag-per-tap of wh).T per tap: [128, 3, 128]
        w4hs = nc.dram_tensor("w4hs", [128, 3, 128], FP16,
                              kind="ExternalInput")
    else:
        wh = nc.dram_tensor("wh", [128, 3], FP32, kind="ExternalInput")
        w4s = nc.dram_tensor("w4s", [128, 128], FP16, kind="ExternalInput")
    bias = nc.dram_tensor("bias", [128, 1], FP32, kind="ExternalInput")
    y = nc.dram_tensor("y", [N_PER_CORE, T_CH, H_IMG, W_IMG], FP16,
                       kind="ExternalOutput")

    # spread the ACT-assigned bias pair ops evenly over the strip
    npairs = GB                     # 2 pairs per c-step, GB/2 c-steps
    n_act = max(0, min(npairs, round(BIAS_ACT * npairs / 8)))
    act_set = {int(i * npairs / n_act) for i in range(n_act)} if n_act else set()

    with TileContext(nc) as tc:
        with (
            tc.tile_pool(name="consts", bufs=1) as consts,
            tc.tile_pool(name="xin", bufs=4) as xin,
            tc.tile_pool(name="mid", bufs=3 if HFOLD else 2) as mid,
            tc.tile_pool(name="h3pool", bufs=1 if HFOLD else 3) as h3pool,
            tc.tile_pool(name="oout", bufs=3) as oout,
            tc.tile_pool(name="psA", bufs=int(os.environ.get("KERNEL_PSA", "2")),
                         space="PSUM") as psumA,
            tc.tile_pool(name="psB", bufs=int(os.environ.get("KERNEL_PSB", "3")),
                         space="PSUM") as psumB,
        ):
            w1vT_t = consts.tile([2 * S_CH, 3, R_CH], FP16)
            bias_t = consts.tile([128, 1], FP32)
            zrow_t = consts.tile([128, 1, W_IMG], FP16)
            nc.gpsimd.memset(zrow_t[:, :, :], 0.0)
            nc.sync.dma_start(out=w1vT_t[:], in_=w1vT[:, :, :])
            nc.sync.dma_start(out=bias_t[:], in_=bias[:, :])
            if HFOLD:
                w4hs_t = consts.tile([128, 3, 128], FP16)
                nc.sync.dma_start(out=w4hs_t[:], in_=w4hs[:, :, :])
            else:
                wh_t = consts.tile([128, 3], FP32)
                w4s_t = consts.tile([128, 128], FP16)
                nc.sync.dma_start(out=wh_t[:], in_=wh[:, :])
                nc.sync.dma_start(out=w4s_t[:], in_=w4s[:, :])

            N_TOT = N_PER_CORE * N_STRIPS
            live = {}

            def load_x(t):
                n, s = divmod(t, N_STRIPS)
                x_t = xin.tile([128, XRN if NOHALO else XR, W_IMG], FP16)
                live[("x", t)] = x_t
                nc.gpsimd.dma_start(out=x_t[:, :, :], in_=xh[n, s, :, :, :])

            def a_step(t, c):
                # ---- stage A chunk-step: 1x1 S->R with the vertical
                # 3x1 depthwise folded in as 3 PSUM-accumulated taps.
                # Groups are col-tiled x4 on the PE; groups 0-1 contract
                # x from partitions 0-63, groups 2-3 from 64-127.
                # Emission is group-major (taps innermost) so each
                # group's start..stop accumulation run is contiguous --
                # a start=True matmul clears written-bits bank-wide, so
                # interleaving groups' taps could drop accumulation.
                x_t = live[("x", t)]
                if c == 0:
                    h2p = mid.tile([128, GB, W_IMG + 2], FP16, tag="h2p",
                                   name=f"h2p_{t}")
                    live[("h2p", t)] = h2p
                    nc.gpsimd.memset(h2p[:, :, 0:1], 0.0)
                    nc.gpsimd.memset(h2p[:, :, W_IMG + 1:W_IMG + 2], 0.0)
                h2p = live[("h2p", t)]
                psA = psumA.tile([128, 2, W_IMG], FP32)
                order = ([(j, kv) for kv in range(3) for j in range(4)]
                         if A_TAPMAJOR else
                         [(j, kv) for j in range(4) for kv in range(3)])
                n_img, s = divmod(t, N_STRIPS)

                def emit(j, kv, rhs, base, drow, nrows):
                    nc.tensor.matmul(
                        psA[32 * j:32 * j + 32, drow:drow + nrows, :],
                        w1vT_t[base:base + S_CH, kv:kv + 1, :], rhs,
                        start=(kv == 0), stop=(kv == 2),
                        tile_position=(base, 32 * j),
                    )

                def src_row(i):
                    # one x row i (strip-local, in [-1, HB]) -> (tile,
                    # partition base, local row); image edges -> zero row
                    if i < 0:
                        if s == 0:
                            return (zrow_t, 64, 0)
                        return (live[("x", t - 1)], 64, XRN - 1)
                    if i >= HB:
                        if s == N_STRIPS - 1:
                            return (zrow_t, 0, 0)
                        return (live[("x", t + 1)], 0, 0)
                    if i < XRN:
                        return (x_t, 0, i)
                    return (x_t, 64, i - XRN)

                for j, kv in order:
                    if not NOHALO:
                        if j < 2:
                            r0 = GB * j + 2 * c + kv
                            rhs = x_t[0:S_CH, r0:r0 + 2, :]
                            base = 0
                        else:
                            r0 = GB * j + 2 * c + kv - 2 * GB
                            rhs = x_t[S_CH:128, r0:r0 + 2, :]
                            base = 64
                        emit(j, kv, rhs, base, 0, 2)
                        continue
                    i = GB * j + 2 * c + kv - 1
                    ta, ba, la = src_row(i)
                    tb, bb, lb = src_row(i + 1)
                    if ta is tb and ba == bb and lb == la + 1:
                        emit(j, kv, ta[ba:ba + S_CH, la:la + 2, :], ba, 0, 2)
                    else:
                        emit(j, kv, ta[ba:ba + S_CH, la:la + 1, :], ba, 0, 1)
                        emit(j, kv, tb[bb:bb + S_CH, lb:lb + 1, :], bb, 1, 1)
                nc.scalar.copy(h2p[:, 2 * c:2 * c + 2, 1:W_IMG + 1],
                               psA[:, :, :])

            def depthwise_h(t):
                # ---- horizontal 1x3 depthwise, batched over the strip.
                # h2p data sits at cols 1..224 with zero pads at 0 and
                # 225, so tap kh reads cols [kh, kh+224).  kh=0/2 are
                # 4B-aligned (fp16 packed modes); the odd-offset kh=1
                # tap is 1x-rate and can run on GpSimd instead.
                if NOHALO:
                    if t >= 1:
                        live.pop(("x", t - 1))
                else:
                    live.pop(("x", t))
                if HFOLD:
                    return
                h2p = live.pop(("h2p", t))
                h3 = h3pool.tile([128, GB, W_IMG], FP16, tag="h3",
                                 name=f"h3_{t}")
                nc.vector.tensor_scalar_mul(
                    h3[:, :, :], h2p[:, :, 0:W_IMG], wh_t[:, 0:1])
                eng1 = nc.gpsimd if GPS_TAP else nc.vector
                eng1.scalar_tensor_tensor(
                    h3[:, :, :], h2p[:, :, 1:W_IMG + 1], wh_t[:, 1:2],
                    h3[:, :, :],
                    op0=mybir.AluOpType.mult, op1=mybir.AluOpType.add)
                nc.vector.scalar_tensor_tensor(
                    h3[:, :, :], h2p[:, :, 2:W_IMG + 2], wh_t[:, 2:3],
                    h3[:, :, :],
                    op0=mybir.AluOpType.mult, op1=mybir.AluOpType.add)
                live[("h3", t)] = h3

            def b_step(t, c):
                # ---- stage B chunk-step: 1x1 R->T row-tiled x4 into
                # 2-bank PSUM pair tiles; one bias+copy op per pair.
                # With HFOLD, each group runs 3 PSUM-accumulated taps
                # whose rhs is h2p column-shifted by kh (the horizontal
                # conv rides the contraction); each pair member owns its
                # own PSUM bank, so interleaved tap emission is safe.
                if c == 0:
                    # [T, group, row-in-group, W] == y rows g*8+r
                    o_t = oout.tile([T_CH, 4, GB, W_IMG], FP16, tag="o_t",
                                    name=f"o_t_{t}")
                    live[("o", t)] = o_t
                o_t = live[("o", t)]
                src = live[("h2p", t)] if HFOLD else live[("h3", t)]
                psBs = [psumB.tile([128, 2, 512], FP32, tag="psB",
                                   name=f"psB_{t}_{c}_{k}")
                        for k in range(2)]
                if HFOLD:
                    # tap-major across all 4 groups: each group owns its
                    # own PSUM bank, so start-flag bit-clears can't
                    # interfere, and consecutive MMs hit independent PE
                    # tiles (pipeline overlap instead of 3-MM chains).
                    for kh in range(3):
                        for g in range(4):
                            nc.tensor.matmul(
                                psBs[g // 2][:, g % 2:g % 2 + 1, 0:2 * W_IMG],
                                w4hs_t[32 * g:32 * g + 32, kh:kh + 1, :],
                                src[32 * g:32 * g + 32, 2 * c:2 * c + 2,
                                    kh:kh + W_IMG],
                                start=(kh == 0), stop=(kh == 2),
                                tile_position=(32 * g, 0),
                            )
                else:
                    for g in range(4):
                        nc.tensor.matmul(
                            psBs[g // 2][:, g % 2:g % 2 + 1, 0:2 * W_IMG],
                            w4s_t[32 * g:32 * g + 32, :],
                            src[32 * g:32 * g + 32, 2 * c:2 * c + 2, :],
                            start=True, stop=True,
                            tile_position=(32 * g, 0),
                        )
                for k2 in range(2):
                    out_ap = o_t[:, 2 * k2:2 * k2 + 2, 2 * c:2 * c + 2, :]
                    in_ap = psBs[k2][:, :, 0:2 * W_IMG]
                    if (2 * c + k2) in act_set:
                        nc.scalar.add(out_ap, in_ap, bias_t[:, 0:1])
                    else:
                        nc.vector.tensor_scalar_add(out_ap, in_ap,
                                                    bias_t[:, 0:1])

            def b_dma(t, half):
                # optionally store rows [0,GB/2) after c-step 1 and the
                # rest after c-step 3, so the drain starts early.
                n, s = divmod(t, N_STRIPS)
                h0 = s * HB
                o_t = live[("o", t)]
                if not SPLIT_STORE and half == 0:
                    return
                # stores ride the scalar HWDGE ring so reads (sync ring)
                # and writes overlap instead of FIFO-ing on one queue
                eng = nc.scalar if t % 2 == 0 else nc.sync
                if SPLIT_STORE:
                    r0 = half * (GB // 2)
                    yv = y[n, :, h0:h0 + HB, :].rearrange(
                        "p (g r) w -> p g r w", g=4)
                    eng.dma_start(
                        out=yv[:, :, r0:r0 + GB // 2, :],
                        in_=o_t[:, :, r0:r0 + GB // 2, :])
                else:
                    eng.dma_start(out=y[n, :, h0:h0 + HB, :],
                                  in_=o_t[:, :, :, :])
                if half == 1:
                    live.pop(("o", t))
                    live.pop(("h2p", t) if HFOLD else ("h3", t))

            # One-strip-pipelined with a SKEW-strip skew for stage B.
            SKEW = 1 if HFOLD else 2
            NC_CH = GB // 2         # 4 chunk-steps for both stages
            for t in range(N_TOT + SKEW):
                if t < N_TOT:
                    if t == 0:
                        load_x(0)
                    if t + 1 < N_TOT:
                        load_x(t + 1)
                    for c in range(NC_CH):
                        if B_FIRST and t >= SKEW:
                            b_step(t - SKEW, c)
                        a_step(t, c)
                        if t >= SKEW:
                            if not B_FIRST:
                                b_step(t - SKEW, c)
                            if c == 1:
                                b_dma(t - SKEW, 0)
                    if t >= SKEW:
                        b_dma(t - SKEW, 1)
                    depthwise_h(t)
                else:
                    for c in range(NC_CH):
                        b_step(t - SKEW, c)
                        if c == 1:
                            b_dma(t - SKEW, 0)
                    b_dma(t - SKEW, 1)

    _legalize_sync(nc)
    return nc


def _prep_weights(s_to_r_weight, depth_vert_weight, depth_hor_weight,
                  r_to_t_weight, r_to_t_bias):
    W1 = s_to_r_weight[:, :, 0, 0].astype(np.float32)          # [32, 64]
    wv = depth_vert_weight[:, 0, :, 0].astype(np.float32)      # [32, 3]
    # lhsT per tap: (diag(wv[:,kv]) @ W1).T = [64, 32]; -> [64, 3, 32]
    w1vT = np.stack([(W1 * wv[:, kv:kv + 1]).T for kv in range(3)],
                    axis=1)
    w1vT = np.ascontiguousarray(
        np.tile(w1vT, (2, 1, 1)).astype(np.float16))           # [128, 3, 32]
    wh = np.ascontiguousarray(
        np.tile(depth_hor_weight[:, 0, 0, :], (4, 1)).astype(np.float32))
    W4T = r_to_t_weight[:, :, 0, 0].T.astype(np.float32)       # [32, 128]
    w4s = np.ascontiguousarray(np.tile(W4T, (4, 1)).astype(np.float16))
    wh32 = depth_hor_weight[:, 0, 0, :].astype(np.float32)     # [32, 3]
    # per-tap (W4 @ diag(wh[:,kh])).T = diag(wh[:,kh]) @ W4.T: [32, 3, 128]
    w4hs = np.stack([W4T * wh32[:, kh:kh + 1] for kh in range(3)], axis=1)
    w4hs = np.ascontiguousarray(
        np.tile(w4hs, (4, 1, 1)).astype(np.float16))           # [128, 3, 128]
    b = np.ascontiguousarray(
        r_to_t_bias.reshape(T_CH, 1).astype(np.float32))
    return w1vT, wh, w4s, w4hs, b


def kernel(x, s_to_r_weight, depth_vert_weight, depth_hor_weight,
           r_to_t_weight, r_to_t_bias):
    global LAST_EXEC_TIME_NS, LAST_RES
    _install_ntff_hook()
    from concourse.bass_utils import run_bass_kernel_spmd

    if "nc" not in _CACHE:
        _CACHE["nc"] = _build_nc()
    nc = _CACHE["nc"]

    x16 = np.asarray(x, dtype=np.float16)
    if NOHALO:
        # pure reshape: [N, S, strip, half, XRN, W] -> [N, strip, 128, XRN, W]
        xh_full = np.ascontiguousarray(
            x16.reshape(N_FULL, S_CH, N_STRIPS, 2, XRN, W_IMG)
               .transpose(0, 2, 3, 1, 4, 5)
               .reshape(N_FULL, N_STRIPS, 128, XRN, W_IMG))
    else:
        # half-major strip tiles with materialized halo rows; edges 0
        xpad = np.zeros((N_FULL, S_CH, H_IMG + 2, W_IMG), dtype=np.float16)
        xpad[:, :, 1:H_IMG + 1] = x16
        sv = np.lib.stride_tricks.sliding_window_view(
            xpad, XR, axis=2)                   # [N,S,H+3-XR,W,XR]
        idx0 = np.arange(N_STRIPS) * HB         # top half: rows h0-1 (+1 pad)
        idx1 = idx0 + 2 * GB                    # bottom half
        h0v = sv[:, :, idx0].transpose(0, 2, 1, 4, 3)
        h1v = sv[:, :, idx1].transpose(0, 2, 1, 4, 3)
        xh_full = np.ascontiguousarray(
            np.concatenate([h0v, h1v], axis=2))  # [N, strip, 128, XR, W]

    w1vT, wh, w4s, w4hs, b = _prep_weights(
        np.asarray(s_to_r_weight), np.asarray(depth_vert_weight),
        np.asarray(depth_hor_weight), np.asarray(r_to_t_weight),
        np.asarray(r_to_t_bias))

    in_maps = []
    for i in range(N_CORES):
        m = {
            "xh": xh_full[i * N_PER_CORE:(i + 1) * N_PER_CORE],
            "w1vT": w1vT, "bias": b,
        }
        if HFOLD:
            m["w4hs"] = w4hs
        else:
            m["wh"] = wh
            m["w4s"] = w4s
        in_maps.append(m)

    trace = bool(int(os.environ.get("KERNEL_TRACE", "0")))
    res = run_bass_kernel_spmd(nc, in_maps, core_ids=list(range(N_CORES)),
                               trace=trace)
    LAST_EXEC_TIME_NS = res.exec_time_ns
    LAST_RES = res

    out = np.empty((N_FULL, T_CH, H_IMG, W_IMG), dtype=np.float32)
    for i in range(N_CORES):
        out[i * N_PER_CORE:(i + 1) * N_PER_CORE] = \
            res.results[i]["y"].astype(np.float32)
    return out
